# revision 36
# baseline (speedup 1.0000x reference)
"""DTNN layer kernel for Trainium2 (8 NeuronCores).

Math: out[b,i,o] = sum_j sum_h Wfc[o,h] * hx[b,i,h] * hd[b,i,j,h]
with hx = x@Wcf.T + bcf, hd = dist@Wdf.T + bdf.
Since Wfc/Wdf are linear, the j-sum commutes:
    ds[b,i,d]  = sum_j dist[b,i,j,d]                  (memory-bound reduction)
    out[b,i,:] = ((x@Wcf.T + bcf) * (ds@Wdf.T + N*bdf)) @ Wfc.T
So the kernel streams `distance` once and reduces it into ds on the fly.

Sharding: flatten (B,N) -> 1024 i-rows, 128 rows per core; no cross-core comms.

v4 design (from NTFF trace analysis of v3):
- v3 was stream-bound: 8.39MB fp16/core at ~374GB/s (per-core HBM roofline)
  = 22.4us, plus an 8us DVE fold tail and a 9us serial startup.
- distance is now cast to fp8 e3m4 on the host (x2 prescale; end-to-end rel
  err ~1.1e-2 vs the 2e-2 gate): stream halves to 4.19MB -> ~11us.
- The PE eats e3m4 blocks directly (1 cyc/row, same as bf16): the j-sum
  rides PSUM accumulation over 128-col block matmuls with wdf (e3m4, x32
  prescale) stationary.  No vector-engine work per block.
- The PE alone would need 14.3us for 256 blocks; the DVE folds ~61 block
  pairs (e3m4+e3m4 -> fp16, exact) so the PE does 195 matmuls (~11us) and
  tracks the stream.  Folded blocks use a fp16 copy of wdf.
- The PE HAM clock gate defaults to 1.2GHz and only reaches 2.4GHz after
  ~3.4us of sustained activity: ~64 small dummy matmuls issued at kernel
  start warm it up during the (fixed) ~7us preamble, so real matmuls run
  at ~56ns instead of ~107ns.
- Constants ride the scalar-engine HWDGE ring, in parallel with the dist
  tiles on the sync-engine ring (each dma_start costs ~700ns of sequencer
  issue time; two rings overlap issue and let the constants land early).
- Dequant scale 1/(ALPHA*GAMMA) is folded into wfc on the host (the bias
  path uses the separate host-folded N*bdf*Wfc matrix, so scales separate).
- Output is written as fp16 and upcast on the host (inside the 2e-2 noise).
"""

import numpy as np
import ml_dtypes

import concourse.bass as bass
import concourse.bacc as bacc
import concourse.mybir as mybir
from concourse.tile import TileContext
from concourse.bass_utils import run_bass_kernel_spmd

B, N, D, H = 4, 256, 128, 128
NCORES = 8
ROWS = B * N // NCORES  # 128 i-rows per core
FP = mybir.dt.float32
F16 = mybir.dt.float16
F8 = mybir.dt.float8e3  # e3m4: 4 mantissa bits

ALPHA = 2.0   # dist prescale into e3m4's [0.25, 15.5] normal range
GAMMA = 32.0  # wdf prescale (|wdf|<0.23 -> x32 keeps most weights normal)

# dist DRAM layout per core: [128 d-partitions, N*ROWS cols], col = j*ROWS + i
# Many small tiles: per-tile completion sems fire every ~1.2us so compute
# chases the slowest DMA engine (engines end-skew by 1-2us run to run)
# tightly instead of waiting on big-tile sems; last tiles tiny+unfolded so
# post-stream serial work is minimal.
SIZES = [16, 48, 64, 56, 16, 8]            # sync-ring j-tiles (j 0..208)
FOLDS = [0, 16, 18, 18, 0, 0]              # j-block PAIRS folded (DVE)
SC_JN = 48                                 # j 208..256: one tile on the
SC_F = 12                                  # scalar ring, issued at start -
                                           # it interleaves with the sync
                                           # stream at ~50% packet share and
                                           # completes mid-stream, so the
                                           # last j-range's work is DONE
                                           # before the straggler engine
                                           # finishes (v13's tail: tile-4
                                           # fold chain ran 4.6us after
                                           # last byte)
FOLD_OP = 8                                # max pairs per DVE fold op (lumpy
                                           # 2.5us monolith folds serialized
                                           # the v6 tail)
NDUM = 130                                 # PE warmup dummies: 130 x 27ns =
                                           # 3.5us continuous > the 3.41us
                                           # HAM window, so the PE clock is
                                           # at 2.4GHz when real work starts
GDUM = 2                                   # gap dummies per tile (keep the
                                           # HAM activity window non-idle)

# cst16 columns (fp16), one transfer, lands before dist tile 0.  The bias
# N*bdf rides as a prescaled row added INTO hd via a rank-1 accumulating
# matmul (hx*(a+b) distributes, and 1/(ALPHA*GAMMA) already sits in wfc),
# so no separate bias matrix / second const transfer is needed.
CA_XT = 0       # x^T                    (128 d, ROWS i)
CA_WCF = 128    # Wcf^T                  (128 d, H)
CA_WDF = 256    # Wdf^T * GAMMA          (128 d, H)   for folded fp16 blocks
CA_WFC = 384    # Wfc^T / (ALPHA*GAMMA)  (128 h, D)
CA_BCF = 512    # partition 0: bcf row (1, H)
CA_ONES = 640   # partition 0: ones row (1, ROWS)
CA_BDFS = 768   # partition 0: ALPHA*GAMMA*N*bdf row (1, H)
CA_WDF8 = 896   # e3m4(Wdf^T * GAMMA) packed as 64 fp16 cols (bitcast on SBUF)
CA_TOT = 960


def build_nc():
    nc = bacc.Bacc("TRN2", target_bir_lowering=False)
    dist8 = nc.declare_dram_parameter("dist8", [128, N * ROWS], F8, isOutput=False)
    cstA = nc.declare_dram_parameter("cstA", [128, CA_TOT], F16, isOutput=False)
    out = nc.declare_dram_parameter("out", [ROWS, D], F16, isOutput=True)

    with TileContext(nc) as tc:
        with (
            tc.tile_pool(name="const", bufs=1) as cpool,
            tc.tile_pool(name="dist", bufs=1) as dpool,
            tc.tile_pool(name="work", bufs=1) as wpool,
            tc.tile_pool(name="psum", bufs=1, space="PSUM") as ppool,
        ):
            # --- Constants ride FIRST on the sync (SP) HWDGE ring so they
            # land before dist tile 0 (in-order per ring; on a second ring
            # they round-robin against dist packets and arrive ~3us late,
            # stalling every matmul - measured in v4).  The wdf e3m4 copy
            # is packed in as raw bytes and bitcast on SBUF.
            cA = cpool.tile([128, CA_TOT], F16)
            nc.sync.dma_start(out=cA[:], in_=cstA[:])
            c8 = cA[:, CA_WDF8:CA_WDF8 + 64].bitcast(F8)

            # Bulk dist tiles on the sync (SP) ring, in order; the final
            # j-range rides the scalar (ACT) ring concurrently.  (Fully
            # alternating tiles across rings was measured MUCH worse - every
            # tile then takes ~2 tile-times - but ONE concurrent tile just
            # shifts its bytes early without moving the stream end.)
            sc_tile = dpool.tile([128, SC_JN * ROWS], F8, tag="distsc")
            nc.scalar.dma_start(
                out=sc_tile[:],
                in_=dist8[:, (N - SC_JN) * ROWS:N * ROWS])
            dtiles = []
            off = 0
            for k, jn in enumerate(SIZES):
                t = dpool.tile([128, jn * ROWS], F8, tag=f"dist{k}")
                nc.sync.dma_start(out=t[:], in_=dist8[:, off * ROWS:(off + jn) * ROWS])
                dtiles.append(t)
                off += jn
            assert off == N - SC_JN

            xT = cA[:, CA_XT:CA_XT + ROWS]
            wcf = cA[:, CA_WCF:CA_WCF + H]
            wdf16 = cA[:, CA_WDF:CA_WDF + H]
            wfc16 = cA[:, CA_WFC:CA_WFC + D]
            bcf_row = cA[0:1, CA_BCF:CA_BCF + H]
            ones_row = cA[0:1, CA_ONES:CA_ONES + ROWS]
            bdfs_row = cA[0:1, CA_BDFS:CA_BDFS + H]

            # --- PE warmup: small dummy matmuls keep the PE HAM window busy
            # through the fixed preamble so real matmuls run at 2.4GHz.
            scr = wpool.tile([128, 32], F16)
            nc.vector.memset(scr[:], 0.0)
            ps_scr = ppool.tile([32, 32], FP)
            for _ in range(NDUM):
                nc.tensor.matmul(ps_scr[:], scr[:], scr[:], start=True, stop=True)

            # --- hx^T = (Wcf^T)^T @ x^T + bcf x ones -> (H, ROWS) PSUM
            hx_ps = ppool.tile([H, ROWS], FP)
            nc.tensor.matmul(hx_ps[:], wcf, xT, start=True, stop=False)
            nc.tensor.matmul(hx_ps[:], bcf_row, ones_row, start=False, stop=True)
            hxT = wpool.tile([H, ROWS], F16)
            nc.vector.tensor_copy(hxT[:], hx_ps[:])
            out_ps = ppool.tile([ROWS, D], FP)

            # --- Streaming j-reduction.
            # Tile k: first 2*f_k blocks are folded pairwise on the DVE in
            # ops of <=FOLD_OP pairs (block b + block b+f_k -> fp16, exact
            # since e3m4 sums fit), the rest go straight to the PE as e3m4
            # matmuls.  Fold matmuls chase their own tile's direct ones so
            # the PE never sits a whole tile behind the DVE.
            hd_ps = ppool.tile([H, ROWS], FP)
            # Processing order (PE and DVE): t0, t1, sc-tile, t2, ... - the
            # scalar-ring tile lands mid-stream, so its work slots in after
            # sync tile 1.
            work = [(dtiles[k], jn, f, k)
                    for k, (jn, f) in enumerate(zip(SIZES, FOLDS))]
            work.insert(2, (sc_tile, SC_JN, SC_F, len(SIZES)))
            ftiles = {}
            fold_ops = {}  # tag -> list of (start_pair, n_pairs)
            for t, jn, f, tag in work:
                if f > 0:
                    ft = wpool.tile([128, f * ROWS], F16, tag=f"fold{tag}")
                    ftiles[tag] = ft
                    ops = []
                    p0 = 0
                    while p0 < f:
                        ops.append((p0, min(FOLD_OP, f - p0)))
                        p0 += min(FOLD_OP, f - p0)
                    fold_ops[tag] = ops
            for t, jn, f, tag in work:
                for p0, np_ in fold_ops.get(tag, []):
                    nc.vector.tensor_add(
                        ftiles[tag][:, p0 * ROWS:(p0 + np_) * ROWS],
                        t[:, p0 * ROWS:(p0 + np_) * ROWS],
                        t[:, (f + p0) * ROWS:(f + p0 + np_) * ROWS],
                    )

            n_mms = 1 + sum(jn - f for _, jn, f, _ in work)
            mi = 0

            def hd_mm(lhs, rhs_ap):
                nonlocal mi
                nc.tensor.matmul(
                    hd_ps[:], lhs, rhs_ap,
                    start=(mi == 0), stop=(mi == n_mms - 1),
                )
                mi += 1

            # First hd matmul: rank-1 bias add, hd += (ALPHA*GAMMA*N*bdf) x
            # ones.  Needs only cst data, so it also opens the group early.
            hd_mm(bdfs_row, ones_row)

            for w, (t, jn, f, tag) in enumerate(work):
                if w > 0:
                    for _ in range(GDUM):
                        nc.tensor.matmul(ps_scr[:], scr[:], scr[:],
                                         start=True, stop=True)
                for b in range(2 * f, jn):
                    hd_mm(c8, t[:, b * ROWS:(b + 1) * ROWS])
                for b in range(f):
                    hd_mm(wdf16, ftiles[tag][:, b * ROWS:(b + 1) * ROWS])
            assert mi == n_mms

            # --- s^T = hd^T * hx^T (one PSUM operand max per DVE op), fp16
            sT = wpool.tile([H, ROWS], F16)
            nc.vector.tensor_mul(sT[:], hd_ps[:], hxT[:])

            # out = s^T^T @ (Wfc^T/(ALPHA*GAMMA))  (bias already inside hd)
            nc.tensor.matmul(out_ps[:], sT[:], wfc16, start=True, stop=True)
            # Tail chain: PSUM->SBUF copy on the (idle) DVE, DMA issue on
            # the untouched scalar ring.
            out_sb = wpool.tile([ROWS, D], F16)
            nc.vector.tensor_copy(out_sb[:], out_ps[:])
            nc.scalar.dma_start(out=out[:], in_=out_sb[:])
    nc.compile()
    return nc


_NC_CACHE = None


def _get_nc():
    global _NC_CACHE
    if _NC_CACHE is None:
        _NC_CACHE = build_nc()
    return _NC_CACHE


def _make_in_maps(x, distance, Wcf_w, Wcf_b, Wdf_w, Wdf_b, Wfc_w):
    x = np.asarray(x, np.float32)
    x_flat = x.reshape(B * N, D)
    # [B*N, N, D] -> e3m4 (x2 prescale) -> [d, j, i_full], then slice per core
    d8 = (np.asarray(distance, np.float32) * ALPHA).astype(ml_dtypes.float8_e3m4)
    dT = np.ascontiguousarray(d8.reshape(B * N, N, D).transpose(2, 1, 0))
    wcfT = np.asarray(Wcf_w, np.float32).T
    wdfT = np.asarray(Wdf_w, np.float32).T
    wfcT = np.asarray(Wfc_w, np.float32).T
    bcf = np.asarray(Wcf_b, np.float32)
    bdf = np.asarray(Wdf_b, np.float32)
    cAblk = np.zeros((128, CA_TOT), np.float16)
    cAblk[:, CA_WCF:CA_WCF + H] = wcfT
    cAblk[:, CA_WDF:CA_WDF + H] = wdfT * GAMMA
    cAblk[:, CA_WFC:CA_WFC + D] = wfcT / (ALPHA * GAMMA)
    cAblk[0, CA_BCF:CA_BCF + H] = bcf
    cAblk[0, CA_ONES:CA_ONES + ROWS] = 1.0
    cAblk[0, CA_BDFS:CA_BDFS + H] = (ALPHA * GAMMA * float(N)) * bdf
    c8blk = (wdfT * GAMMA).astype(ml_dtypes.float8_e3m4)
    cAblk[:, CA_WDF8:CA_WDF8 + 64] = (
        np.ascontiguousarray(c8blk).view(np.uint8).reshape(128, H)
        .view(np.uint16).view(np.float16)
    )
    in_maps = []
    for c in range(NCORES):
        sl = slice(c * ROWS, (c + 1) * ROWS)
        cb = cAblk.copy()
        cb[:, CA_XT:CA_XT + ROWS] = x_flat[sl].T
        in_maps.append({
            "dist8": np.ascontiguousarray(dT[:, :, sl]).reshape(128, N * ROWS),
            "cstA": cb,
        })
    return in_maps


def kernel(x, distance, Wcf_w, Wcf_b, Wdf_w, Wdf_b, Wfc_w):
    in_maps = _make_in_maps(x, distance, Wcf_w, Wcf_b, Wdf_w, Wdf_b, Wfc_w)
    nc = _get_nc()
    res = run_bass_kernel_spmd(nc, in_maps, list(range(NCORES))).results
    out = np.concatenate(
        [res[c]["out"].astype(np.float32) for c in range(NCORES)], axis=0
    )
    return out.reshape(B, N, D)


# revision 39
# speedup vs baseline: 1.0244x; 1.0244x over previous
"""DTNN layer kernel for Trainium2 (8 NeuronCores).

Math: out[b,i,o] = sum_j sum_h Wfc[o,h] * hx[b,i,h] * hd[b,i,j,h]
with hx = x@Wcf.T + bcf, hd = dist@Wdf.T + bdf.
Since Wfc/Wdf are linear, the j-sum commutes:
    ds[b,i,d]  = sum_j dist[b,i,j,d]                  (memory-bound reduction)
    out[b,i,:] = ((x@Wcf.T + bcf) * (ds@Wdf.T + N*bdf)) @ Wfc.T
So the kernel streams `distance` once and reduces it into ds on the fly.

Sharding: flatten (B,N) -> 1024 i-rows, 128 rows per core; no cross-core comms.

v4 design (from NTFF trace analysis of v3):
- v3 was stream-bound: 8.39MB fp16/core at ~374GB/s (per-core HBM roofline)
  = 22.4us, plus an 8us DVE fold tail and a 9us serial startup.
- distance is now cast to fp8 e3m4 on the host (x2 prescale; end-to-end rel
  err ~1.1e-2 vs the 2e-2 gate): stream halves to 4.19MB -> ~11us.
- The PE eats e3m4 blocks directly (1 cyc/row, same as bf16): the j-sum
  rides PSUM accumulation over 128-col block matmuls with wdf (e3m4, x32
  prescale) stationary.  No vector-engine work per block.
- The PE alone would need 14.3us for 256 blocks; the DVE folds ~61 block
  pairs (e3m4+e3m4 -> fp16, exact) so the PE does 195 matmuls (~11us) and
  tracks the stream.  Folded blocks use a fp16 copy of wdf.
- The PE HAM clock gate defaults to 1.2GHz and only reaches 2.4GHz after
  ~3.4us of sustained activity: ~64 small dummy matmuls issued at kernel
  start warm it up during the (fixed) ~7us preamble, so real matmuls run
  at ~56ns instead of ~107ns.
- Constants ride the scalar-engine HWDGE ring, in parallel with the dist
  tiles on the sync-engine ring (each dma_start costs ~700ns of sequencer
  issue time; two rings overlap issue and let the constants land early).
- Dequant scale 1/(ALPHA*GAMMA) is folded into wfc on the host (the bias
  path uses the separate host-folded N*bdf*Wfc matrix, so scales separate).
- Output is written as fp16 and upcast on the host (inside the 2e-2 noise).
"""

import numpy as np
import ml_dtypes

import concourse.bass as bass
import concourse.bacc as bacc
import concourse.mybir as mybir
from concourse.tile import TileContext
from concourse.bass_utils import run_bass_kernel_spmd

B, N, D, H = 4, 256, 128, 128
NCORES = 8
ROWS = B * N // NCORES  # 128 i-rows per core
FP = mybir.dt.float32
F16 = mybir.dt.float16
F8 = mybir.dt.float8e3  # e3m4: 4 mantissa bits

ALPHA = 2.0   # dist prescale into e3m4's [0.25, 15.5] normal range
GAMMA = 32.0  # wdf prescale (|wdf|<0.23 -> x32 keeps most weights normal)

# dist DRAM layout per core: [128 d-partitions, N*ROWS cols], col = j*ROWS + i
# Many small tiles: per-tile completion sems fire every ~1.2us so compute
# chases the slowest DMA engine (engines end-skew by 1-2us run to run)
# tightly instead of waiting on big-tile sems; last tiles tiny+unfolded so
# post-stream serial work is minimal.
SIZES = [16, 48, 64, 64, 48, 12, 4]
FOLDS = [0, 16, 18, 18, 12, 0, 0]          # j-block PAIRS folded (DVE)
FOLD_OP = 8                                # max pairs per DVE fold op (lumpy
                                           # 2.5us monolith folds serialized
                                           # the v6 tail)
NDUM = 130                                 # PE warmup dummies: 130 x 27ns =
                                           # 3.5us continuous > the 3.41us
                                           # HAM window, so the PE clock is
                                           # at 2.4GHz when real work starts
GDUM = 2                                   # gap dummies per tile (keep the
                                           # HAM activity window non-idle)

# cst16 columns (fp16), one transfer, lands before dist tile 0.  The bias
# N*bdf rides as a prescaled row added INTO hd via a rank-1 accumulating
# matmul (hx*(a+b) distributes, and 1/(ALPHA*GAMMA) already sits in wfc),
# so no separate bias matrix / second const transfer is needed.
CA_XT = 0       # x^T                    (128 d, ROWS i)
CA_WCF = 128    # Wcf^T                  (128 d, H)
CA_WDF = 256    # Wdf^T * GAMMA          (128 d, H)   for folded fp16 blocks
CA_WFC = 384    # Wfc^T / (ALPHA*GAMMA)  (128 h, D)
CA_BCF = 512    # partition 0: bcf row (1, H)
CA_ONES = 640   # partition 0: ones row (1, ROWS)
CA_BDFS = 768   # partition 0: ALPHA*GAMMA*N*bdf row (1, H)
CA_WDF8 = 896   # e3m4(Wdf^T * GAMMA) packed as 64 fp16 cols (bitcast on SBUF)
CA_TOT = 960


def build_nc():
    nc = bacc.Bacc("TRN2", target_bir_lowering=False)
    dist8 = nc.declare_dram_parameter("dist8", [128, N * ROWS], F8, isOutput=False)
    cstA = nc.declare_dram_parameter("cstA", [128, CA_TOT], F16, isOutput=False)
    out = nc.declare_dram_parameter("out", [ROWS, D], F16, isOutput=True)

    with TileContext(nc) as tc:
        with (
            tc.tile_pool(name="const", bufs=1) as cpool,
            tc.tile_pool(name="dist", bufs=1) as dpool,
            tc.tile_pool(name="work", bufs=1) as wpool,
            tc.tile_pool(name="psum", bufs=1, space="PSUM") as ppool,
        ):
            # --- Constants ride FIRST on the sync (SP) HWDGE ring so they
            # land before dist tile 0 (in-order per ring; on a second ring
            # they round-robin against dist packets and arrive ~3us late,
            # stalling every matmul - measured in v4).  The wdf e3m4 copy
            # is packed in as raw bytes and bitcast on SBUF.
            cA = cpool.tile([128, CA_TOT], F16)
            nc.sync.dma_start(out=cA[:], in_=cstA[:])
            c8 = cA[:, CA_WDF8:CA_WDF8 + 64].bitcast(F8)

            # All dist tiles on the sync (SP) ring, in order.  (Interleaving
            # the two HWDGE rings was measured MUCH worse: engines
            # round-robin rings at packet granularity, so every tile takes
            # ~2 tile-times to complete and each sem lands a tile late.)
            dtiles = []
            off = 0
            for k, jn in enumerate(SIZES):
                t = dpool.tile([128, jn * ROWS], F8, tag=f"dist{k}")
                nc.sync.dma_start(out=t[:], in_=dist8[:, off * ROWS:(off + jn) * ROWS])
                dtiles.append(t)
                off += jn

            xT = cA[:, CA_XT:CA_XT + ROWS]
            wcf = cA[:, CA_WCF:CA_WCF + H]
            wdf16 = cA[:, CA_WDF:CA_WDF + H]
            wfc16 = cA[:, CA_WFC:CA_WFC + D]
            bcf_row = cA[0:1, CA_BCF:CA_BCF + H]
            ones_row = cA[0:1, CA_ONES:CA_ONES + ROWS]
            bdfs_row = cA[0:1, CA_BDFS:CA_BDFS + H]

            # --- PE warmup: small dummy matmuls keep the PE HAM window busy
            # through the fixed preamble so real matmuls run at 2.4GHz.
            scr = wpool.tile([128, 32], F16)
            nc.vector.memset(scr[:], 0.0)
            ps_scr = ppool.tile([32, 32], FP)
            for _ in range(NDUM):
                nc.tensor.matmul(ps_scr[:], scr[:], scr[:], start=True, stop=True)

            # --- hx^T = (Wcf^T)^T @ x^T + bcf x ones -> (H, ROWS) PSUM
            hx_ps = ppool.tile([H, ROWS], FP)
            nc.tensor.matmul(hx_ps[:], wcf, xT, start=True, stop=False)
            nc.tensor.matmul(hx_ps[:], bcf_row, ones_row, start=False, stop=True)
            hxT = wpool.tile([H, ROWS], F16)
            nc.vector.tensor_copy(hxT[:], hx_ps[:])
            out_ps = ppool.tile([ROWS, D], FP)

            # --- Streaming j-reduction.
            # Tile k: first 2*f_k blocks are folded pairwise on the DVE in
            # ops of <=FOLD_OP pairs (block b + block b+f_k -> fp16, exact
            # since e3m4 sums fit), the rest go straight to the PE as e3m4
            # matmuls.  Fold matmuls chase their own tile's direct ones so
            # the PE never sits a whole tile behind the DVE.
            hd_ps = ppool.tile([H, ROWS], FP)
            ftiles = []
            fold_ops = []  # per tile: list of (start_pair, n_pairs)
            for k, (jn, f) in enumerate(zip(SIZES, FOLDS)):
                if f > 0:
                    ft = wpool.tile([128, f * ROWS], F16, tag=f"fold{k}")
                    ftiles.append(ft)
                    ops = []
                    p0 = 0
                    while p0 < f:
                        ops.append((p0, min(FOLD_OP, f - p0)))
                        p0 += min(FOLD_OP, f - p0)
                    fold_ops.append(ops)
                else:
                    ftiles.append(None)
                    fold_ops.append([])
            for k, (jn, f) in enumerate(zip(SIZES, FOLDS)):
                for p0, np_ in fold_ops[k]:
                    nc.vector.tensor_add(
                        ftiles[k][:, p0 * ROWS:(p0 + np_) * ROWS],
                        dtiles[k][:, p0 * ROWS:(p0 + np_) * ROWS],
                        dtiles[k][:, (f + p0) * ROWS:(f + p0 + np_) * ROWS],
                    )

            n_mms = 1 + sum(jn - f for jn, f in zip(SIZES, FOLDS))
            mi = 0

            def hd_mm(lhs, rhs_ap):
                nonlocal mi
                nc.tensor.matmul(
                    hd_ps[:], lhs, rhs_ap,
                    start=(mi == 0), stop=(mi == n_mms - 1),
                )
                mi += 1

            # First hd matmul: rank-1 bias add, hd += (ALPHA*GAMMA*N*bdf) x
            # ones.  Needs only cst data, so it also opens the group early.
            hd_mm(bdfs_row, ones_row)

            last_fold = max(k for k, f in enumerate(FOLDS) if f > 0)
            for k, (jn, f) in enumerate(zip(SIZES, FOLDS)):
                t = dtiles[k]
                if k > 0:
                    for _ in range(GDUM):
                        nc.tensor.matmul(ps_scr[:], scr[:], scr[:],
                                         start=True, stop=True)
                for b in range(2 * f, jn):
                    hd_mm(c8, t[:, b * ROWS:(b + 1) * ROWS])
                if k == last_fold:
                    continue  # deferred: see below
                for b in range(f):
                    hd_mm(wdf16, ftiles[k][:, b * ROWS:(b + 1) * ROWS])
            # The LAST folded tile's fold-matmuls go after the final direct
            # tiles: its DVE fold only starts at that tile's (straggler-
            # gated) sem, and placed in tile order it blocks the later
            # tiles' direct matmuls in the PE FIFO for ~1us while the fold
            # finishes; emitted here, the fold latency hides behind direct
            # work whose data already arrived.
            for b in range(FOLDS[last_fold]):
                hd_mm(wdf16, ftiles[last_fold][:, b * ROWS:(b + 1) * ROWS])
            assert mi == n_mms

            # --- s^T = hd^T * hx^T (one PSUM operand max per DVE op), fp16
            sT = wpool.tile([H, ROWS], F16)
            nc.vector.tensor_mul(sT[:], hd_ps[:], hxT[:])

            # out = s^T^T @ (Wfc^T/(ALPHA*GAMMA))  (bias already inside hd)
            nc.tensor.matmul(out_ps[:], sT[:], wfc16, start=True, stop=True)
            # Tail chain: PSUM->SBUF copy on the (idle) DVE, DMA issue on
            # the sync ring (long drained; SP's sequencer is ~100ns faster
            # than ACT's).
            out_sb = wpool.tile([ROWS, D], F16)
            nc.vector.tensor_copy(out_sb[:], out_ps[:])
            nc.sync.dma_start(out=out[:], in_=out_sb[:])
    nc.compile()
    return nc


_NC_CACHE = None


def _get_nc():
    global _NC_CACHE
    if _NC_CACHE is None:
        _NC_CACHE = build_nc()
    return _NC_CACHE


def _make_in_maps(x, distance, Wcf_w, Wcf_b, Wdf_w, Wdf_b, Wfc_w):
    x = np.asarray(x, np.float32)
    x_flat = x.reshape(B * N, D)
    # [B*N, N, D] -> e3m4 (x2 prescale) -> [d, j, i_full], then slice per core
    d8 = (np.asarray(distance, np.float32) * ALPHA).astype(ml_dtypes.float8_e3m4)
    dT = np.ascontiguousarray(d8.reshape(B * N, N, D).transpose(2, 1, 0))
    wcfT = np.asarray(Wcf_w, np.float32).T
    wdfT = np.asarray(Wdf_w, np.float32).T
    wfcT = np.asarray(Wfc_w, np.float32).T
    bcf = np.asarray(Wcf_b, np.float32)
    bdf = np.asarray(Wdf_b, np.float32)
    cAblk = np.zeros((128, CA_TOT), np.float16)
    cAblk[:, CA_WCF:CA_WCF + H] = wcfT
    cAblk[:, CA_WDF:CA_WDF + H] = wdfT * GAMMA
    cAblk[:, CA_WFC:CA_WFC + D] = wfcT / (ALPHA * GAMMA)
    cAblk[0, CA_BCF:CA_BCF + H] = bcf
    cAblk[0, CA_ONES:CA_ONES + ROWS] = 1.0
    cAblk[0, CA_BDFS:CA_BDFS + H] = (ALPHA * GAMMA * float(N)) * bdf
    c8blk = (wdfT * GAMMA).astype(ml_dtypes.float8_e3m4)
    cAblk[:, CA_WDF8:CA_WDF8 + 64] = (
        np.ascontiguousarray(c8blk).view(np.uint8).reshape(128, H)
        .view(np.uint16).view(np.float16)
    )
    in_maps = []
    for c in range(NCORES):
        sl = slice(c * ROWS, (c + 1) * ROWS)
        cb = cAblk.copy()
        cb[:, CA_XT:CA_XT + ROWS] = x_flat[sl].T
        in_maps.append({
            "dist8": np.ascontiguousarray(dT[:, :, sl]).reshape(128, N * ROWS),
            "cstA": cb,
        })
    return in_maps


def kernel(x, distance, Wcf_w, Wcf_b, Wdf_w, Wdf_b, Wfc_w):
    in_maps = _make_in_maps(x, distance, Wcf_w, Wcf_b, Wdf_w, Wdf_b, Wfc_w)
    nc = _get_nc()
    res = run_bass_kernel_spmd(nc, in_maps, list(range(NCORES))).results
    out = np.concatenate(
        [res[c]["out"].astype(np.float32) for c in range(NCORES)], axis=0
    )
    return out.reshape(B, N, D)
